# revision 13
# baseline (speedup 1.0000x reference)
"""AttentionPool Trainium2 kernel (8-core SPMD, batch-sharded).

Math (algebraically folded from the reference):
  The single learned query collapses attention to:
    ws[h,:]   = sum_{d in head h} q_flat[h*64+d] * wk[h*64+d, :]   (host, tiny)
    s[b,h,n]  = tokens[b,n,:] @ (ws[h,:] * scale)                  (device)
    p         = exp(s)            (softmax shift cancels; |s| <~ 2, fp32-safe)
    pooled    = (p @ tokens) / sum_n p                             (device)
    ctx[b,hd] = wv[hd,:] @ pooled[b,h,:] ;  out = ctx @ out_w.T + c
  Per-head score bias is a constant shift within each softmax row and cancels
  exactly; all other biases fold into c = out_w @ bv + out_b (host).

Device per core: stream its 4 batches once as TWO copies — natural [n,d] in
fp16 (for the pooled matmul) and transposed [d,n] in fp8e4 (for the scores
matmul; fp8 verified to 6.5e-3 rel err end-to-end, and the PE accepts a
fp16-stationary x fp8-moving matmul) — 37.8 MB total. 1024-token tiles, one
DMA per stream per tile (split across the sync and scalar HWDGE rings).
Scores via fp16 ws x fp8 tokT matmuls, exp on ACT with accumulated row sums,
pooled as a PSUM-accumulated matmul against the natural tile. Per-batch
normalize + transpose overlap the next batch's stream; the final projections
are 36+14 matmuls straight into a [bloc, D] PSUM tile (no output transpose).
"""

import numpy as np

P = 128
D = 768
H = 12
DH = 64
DJ = D // P          # 6 chunks of the model dim
C = 1024             # tokens per tile
S = C // P           # 8 sub-chunks of 128 tokens
B = 32
N = 4096
NCORES = 8
BLOC = B // NCORES   # batches per core

_PATCHED = False


def _patch_tile_drain():
    """This walrus build allows only ONE sync wait per instruction (2 for
    EventSemaphore), but TileContext._drain_and_barrier puts a wait per
    outstanding semaphore on the single tail Drain. Split: one Drain each."""
    global _PATCHED
    if _PATCHED:
        return
    import bass_rust
    import concourse.tile as tile
    from concourse.vector_clock import ScopedClock

    def _drain_and_barrier(self, tick_clock, wait_clock):
        nc = self.nc
        probe = nc.sync.drain()
        wait_clock.add_sem_waits(
            probe.ins, ScopedClock({None: tick_clock.global_clock})
        )
        si = probe.ins.sync_info
        if si is not None and len(si.on_wait) > 1:
            waits = list(si.on_wait)
            probe.ins.sync_info = bass_rust.SyncInfo(
                on_wait=[waits[0]], on_update=list(si.on_update)
            )
            for w in waits[1:]:
                extra = nc.sync.drain()
                extra.ins.sync_info = bass_rust.SyncInfo(on_wait=[w], on_update=[])
        nc.all_engine_barrier()
        popped = nc._tile_sem_poison_stack.pop()
        assert popped is self._sem_poison
        nc.clear_and_free_semaphores(list(self.sems.allocated().values()))
        nc.all_engine_barrier()

    tile.TileContext._drain_and_barrier = _drain_and_barrier
    _PATCHED = True


def _legalize_waits(nc):
    """TRN2 walrus encodes at most ONE sync wait per instruction (two for
    EventSemaphore). Tile's wait assignment can leave more; hoist the extras
    onto standalone EventSemaphore instructions inserted just before, on the
    same engine (same semantics: engine blocks on them in order)."""
    import bass_rust
    from concourse import mybir

    n_fixed = 0
    for f in nc.m.functions:
        for bb in f.blocks:
            out = []
            for inst in bb.instructions:
                si = inst.sync_info
                waits = list(si.on_wait) if si is not None else []
                cap = 2 if isinstance(inst, mybir.InstEventSemaphore) else 1
                if len(waits) > cap:
                    extras, keep = waits[:-cap], waits[-cap:]
                    for i in range(0, len(extras), 2):
                        ev = mybir.InstEventSemaphore(
                            name=f"EVW-{inst.name}-{i}", ins=[], outs=[]
                        )
                        ev.engine = inst.engine
                        ev.sync_info = bass_rust.SyncInfo(
                            on_wait=extras[i : i + 2], on_update=[]
                        )
                        out.append(ev)
                    inst.sync_info = bass_rust.SyncInfo(
                        on_wait=keep, on_update=list(si.on_update)
                    )
                    n_fixed += 1
                out.append(inst)
            bb.instructions = out
    return n_fixed


def build_nc(bloc=BLOC, n=N, legalize=True):
    import concourse.bass as bass
    import concourse.tile as tile
    from concourse import mybir
    from concourse.masks import make_identity

    f32 = mybir.dt.float32
    f16 = mybir.dt.float16
    f8 = mybir.dt.float8e4
    EXP = mybir.ActivationFunctionType.Exp
    nt = n // C

    nc = bass.Bass()
    tokens = nc.declare_dram_parameter("tokens", [bloc, n, D], f16, isOutput=False)
    tokT8 = nc.declare_dram_parameter("tokT8", [bloc, D, n], f8, isOutput=False)
    wsT = nc.declare_dram_parameter("wsT", [DJ, P, H], f16, isOutput=False)
    wvT = nc.declare_dram_parameter("wvT", [DJ, P, D], f16, isOutput=False)
    owT = nc.declare_dram_parameter("owT", [DJ, P, D], f16, isOutput=False)
    cvec = nc.declare_dram_parameter("cvec", [1, D], f16, isOutput=False)
    ones = nc.declare_dram_parameter("ones", [1, bloc], f16, isOutput=False)
    out_d = nc.declare_dram_parameter("out", [bloc, D], f32, isOutput=True)

    tokens_ap = tokens[:, :, :]
    tokT8_ap = tokT8[:, :, :]

    with tile.TileContext(nc) as tc:
        with (
            tc.tile_pool(name="singles", bufs=1) as singles,
            tc.tile_pool(name="tok", bufs=4) as tok_pool,
            tc.tile_pool(name="tokT", bufs=4) as tokT_pool,
            tc.tile_pool(name="pp", bufs=2) as p_pool,
            tc.tile_pool(name="lp", bufs=2) as lp_pool,
            tc.tile_pool(name="pn", bufs=2) as pn_pool,
            tc.tile_pool(name="scps", bufs=4, space="PSUM") as sc_psum,
            tc.tile_pool(name="ptps", bufs=2, space="PSUM") as pt_psum,
            tc.tile_pool(name="pops", bufs=1, space="PSUM") as pooled_psum,
        ):
            # tile schedule: 512-token tiles at the global start (shorter
            # pipeline fill) and end (shorter exposed drain chain), 1024
            # in the middle
            def batch_sizes(b):
                if b == 0:
                    return [512, 512] + [1024] * ((n - 1024) // C)
                if b == bloc - 1:
                    return [1024] * ((n - 1024) // C) + [512, 512]
                return [1024] * (n // C)

            flat = []
            for b in range(bloc):
                pos = 0
                sizes = batch_sizes(b)
                for k, sz in enumerate(sizes):
                    flat.append((b, pos, sz, k == 0, k == len(sizes) - 1))
                    pos += sz

            def issue_tile_dmas(key):
                b, n0, sz, _, _ = key
                tok_h = tok_pool.tile([P, sz // P, D], f16, tag="tok")
                nc.sync.dma_start(
                    out=tok_h,
                    in_=tokens_ap[b, n0 : n0 + sz, :].rearrange(
                        "(s p) d -> p s d", p=P
                    ),
                )
                tokT = tokT_pool.tile([P, DJ, sz], f8, tag="tokT")
                nc.scalar.dma_start(
                    out=tokT,
                    in_=tokT8_ap[b, :, n0 : n0 + sz].rearrange(
                        "(j p) n -> p j n", p=P
                    ),
                )
                return tok_h, tokT

            # issue the first tiles' token DMAs before any setup so HBM
            # streaming starts as early as possible
            dmas = {}
            for key in flat[:4]:
                dmas[key] = issue_tile_dmas(key)

            ident = singles.tile([P, P], f32)
            make_identity(nc, ident)
            ident_h = singles.tile([P, P], f16)
            nc.vector.tensor_copy(out=ident_h, in_=ident)
            wsT_sb = singles.tile([P, DJ, H], f16)
            nc.gpsimd.dma_start(
                out=wsT_sb, in_=wsT[:, :, :].rearrange("j p h -> p j h")
            )
            wvT_sb = singles.tile([P, DJ, D], f16)
            nc.gpsimd.dma_start(
                out=wvT_sb, in_=wvT[:, :, :].rearrange("j p d -> p j d")
            )
            owT_sb = singles.tile([P, DJ, D], f16)
            nc.gpsimd.dma_start(
                out=owT_sb, in_=owT[:, :, :].rearrange("j p d -> p j d")
            )
            cvec_sb = singles.tile([1, D], f16)
            nc.gpsimd.dma_start(out=cvec_sb, in_=cvec[:, :])
            ones_sb = singles.tile([1, bloc], f16)
            nc.gpsimd.dma_start(out=ones_sb, in_=ones[:, :])
            l_acc = singles.tile([H, bloc], f32)
            linv = singles.tile([H, bloc], f32)
            pstack = singles.tile([P, DJ, H, bloc], f16)

            pooled_tiles = {}

            def tile_block(key, tok_tiles):
                """Full per-tile compute, in 512-token halves so the exp of
                one half hides under the scores matmuls of the next and the
                PE never stalls on the ACT latency."""
                b, n0, sz, first, last = key
                tok_h, tokT = tok_tiles
                nhalves = sz // 512
                if first:
                    pooled_tiles[b] = pooled_psum.tile(
                        [H, D], f32, tag="po", name=f"pooled_b{b}"
                    )
                pooled_ps = pooled_tiles[b]
                p_halves = []
                for half in range(nhalves):
                    sl = slice(half * 512, (half + 1) * 512)
                    ps = sc_psum.tile([H, 512], f32, tag="sc")
                    for j in range(DJ):
                        nc.tensor.matmul(
                            ps,
                            wsT_sb[:, j, :],
                            tokT[:, j, sl],
                            start=(j == 0),
                            stop=(j == DJ - 1),
                        )
                    p_t = p_pool.tile([H, 512], f16, tag="p")
                    lp = lp_pool.tile([H, 1], f32, tag="l")
                    nc.scalar.activation(out=p_t, in_=ps, func=EXP, accum_out=lp)
                    if first and half == 0:
                        nc.vector.tensor_copy(out=l_acc[:, b : b + 1], in_=lp)
                    else:
                        nc.vector.tensor_add(
                            out=l_acc[:, b : b + 1], in0=l_acc[:, b : b + 1], in1=lp
                        )
                    p_halves.append(p_t)
                for half in range(nhalves):
                    p_t = p_halves[half]
                    pT_ps = pt_psum.tile([P, 4 * H], f16, tag="pt")
                    for s in range(4):
                        nc.tensor.transpose(
                            pT_ps[:, s * H : (s + 1) * H],
                            p_t[:, s * P : (s + 1) * P],
                            ident_h[:H, :H],
                        )
                    pT = p_pool.tile([P, 4 * H], f16, tag="pT")
                    nc.vector.tensor_copy(out=pT, in_=pT_ps)
                    for s in range(4):
                        st = first and half == 0 and s == 0
                        sp = last and half == nhalves - 1 and s == 3
                        sg = half * 4 + s
                        nc.tensor.matmul(
                            pooled_ps[:, 0:512],
                            pT[:, s * H : (s + 1) * H],
                            tok_h[:, sg, 0:512],
                            start=st,
                            stop=sp,
                        )
                        nc.tensor.matmul(
                            pooled_ps[:, 512:768],
                            pT[:, s * H : (s + 1) * H],
                            tok_h[:, sg, 512:768],
                            start=st,
                            stop=sp,
                        )
                if last:
                    # normalize this batch + transpose into pstack
                    nc.vector.reciprocal(linv[:, b : b + 1], l_acc[:, b : b + 1])
                    pooled_n = pn_pool.tile([H, D], f16, tag="pn")
                    nc.vector.tensor_scalar_mul(
                        pooled_n, pooled_ps, linv[:, b : b + 1]
                    )
                    trp = pt_psum.tile([P, DJ * H], f16, tag="pt")
                    for j in range(DJ):
                        nc.tensor.transpose(
                            trp[:, j * H : (j + 1) * H],
                            pooled_n[:, j * P : (j + 1) * P],
                            ident_h[:H, :H],
                        )
                    nc.vector.tensor_copy(
                        out=pstack[:, :, :, b],
                        in_=trp[:, :].rearrange("p (j h) -> p j h", h=H),
                    )

            for key in flat:
                if key not in dmas:
                    dmas[key] = issue_tile_dmas(key)
                tile_block(key, dmas.pop(key))

            # ---- tail: ctx = wv-projection (select 2 heads per e-block),
            # then out = ctx-blocks^T @ ow^T + bias, directly in [bloc, D]
            ctx_sb = singles.tile([P, DJ, bloc], f16)
            for e in range(DJ):
                po = pt_psum.tile([P, H * bloc], f32, tag="pt")
                for j in range(DJ):
                    nc.tensor.matmul(
                        po,
                        wvT_sb[:, j, e * P : (e + 1) * P],
                        pstack[:, j, :, :],
                        start=(j == 0),
                        stop=(j == DJ - 1),
                    )
                h0, h1 = 2 * e, 2 * e + 1
                nc.vector.tensor_copy(
                    out=ctx_sb[0:DH, e, :], in_=po[0:DH, h0 * bloc : (h0 + 1) * bloc]
                )
                nc.vector.tensor_copy(
                    out=ctx_sb[DH:P, e, :], in_=po[DH:P, h1 * bloc : (h1 + 1) * bloc]
                )
            out_ps = pooled_psum.tile([bloc, D], f32, tag="po")
            for half, sl in ((0, slice(0, 512)), (1, slice(512, 768))):
                nc.tensor.matmul(
                    out_ps[:, sl], ones_sb, cvec_sb[:, sl], start=True, stop=False
                )
                for e in range(DJ):
                    nc.tensor.matmul(
                        out_ps[:, sl],
                        ctx_sb[:, e, :],
                        owT_sb[:, e, sl],
                        start=False,
                        stop=(e == DJ - 1),
                    )
            fin_sb = singles.tile([bloc, D], f32)
            nc.vector.tensor_copy(out=fin_sb, in_=out_ps)
            nc.sync.dma_start(out=out_d[:, :], in_=fin_sb)
    if legalize:
        _legalize_waits(nc)
    return nc


def host_prep(query, in_proj_w, in_proj_b, out_w, out_b):
    scale = 1.0 / np.sqrt(DH)
    wq, wk = in_proj_w[:D], in_proj_w[D : 2 * D]
    wv = in_proj_w[2 * D :]
    bq = in_proj_b[:D]
    bv = in_proj_b[2 * D :]
    q_flat = query[0, 0] @ wq.T + bq
    ws = (q_flat.reshape(H, DH)[:, :, None] * wk.reshape(H, DH, D)).sum(1)
    ws_scaled = (ws * scale).astype(np.float32)
    wsT_r = np.ascontiguousarray(ws_scaled.T.astype(np.float16)).reshape(DJ, P, H)
    wvT_r = np.ascontiguousarray(wv.T.astype(np.float16)).reshape(DJ, P, D)
    owT_r = np.ascontiguousarray(out_w.T.astype(np.float16)).reshape(DJ, P, D)
    cvec_r = (out_w @ bv + out_b).astype(np.float16).reshape(1, D)
    ones_r = np.ones((1, BLOC), dtype=np.float16)
    return wsT_r, wvT_r, owT_r, cvec_r, ones_r


def make_in_maps(tokens):
    import ml_dtypes

    tok16 = tokens.astype(np.float16)
    tok8T = np.ascontiguousarray(tokens.transpose(0, 2, 1)).astype(
        ml_dtypes.float8_e4m3
    )
    return tok16, tok8T


def kernel(tokens, query, in_proj_w, in_proj_b, out_w, out_b):
    _patch_tile_drain()
    from concourse.bass_utils import run_bass_kernel_spmd

    tokens = np.asarray(tokens, dtype=np.float32)
    query = np.asarray(query, dtype=np.float32)
    in_proj_w = np.asarray(in_proj_w, dtype=np.float32)
    in_proj_b = np.asarray(in_proj_b, dtype=np.float32)
    out_w = np.asarray(out_w, dtype=np.float32)
    out_b = np.asarray(out_b, dtype=np.float32)

    wsT_r, wvT_r, owT_r, cvec_r, ones_r = host_prep(
        query, in_proj_w, in_proj_b, out_w, out_b
    )
    nc = build_nc()
    tok16, tok8T = make_in_maps(tokens)
    in_maps = [
        {
            "tokens": np.ascontiguousarray(tok16[i * BLOC : (i + 1) * BLOC]),
            "tokT8": tok8T[i * BLOC : (i + 1) * BLOC],
            "wsT": wsT_r,
            "wvT": wvT_r,
            "owT": owT_r,
            "cvec": cvec_r,
            "ones": ones_r,
        }
        for i in range(NCORES)
    ]
    res = run_bass_kernel_spmd(nc, in_maps, core_ids=list(range(NCORES)))
    return np.concatenate(
        [res.results[i]["out"] for i in range(NCORES)], axis=0
    ).astype(np.float32)


# revision 14
# speedup vs baseline: 1.1162x; 1.1162x over previous
"""AttentionPool Trainium2 kernel (8-core SPMD, batch-sharded).

Math (algebraically folded from the reference):
  The single learned query collapses attention to:
    ws[h,:]   = sum_{d in head h} q_flat[h*64+d] * wk[h*64+d, :]   (host, tiny)
    s[b,h,n]  = tokens[b,n,:] @ (ws[h,:] * scale)                  (device)
    p         = exp(s)            (softmax shift cancels; |s| <~ 2, fp32-safe)
    pooled    = (p @ tokens) / sum_n p                             (device)
    ctx[b,hd] = wv[hd,:] @ pooled[b,h,:] ;  out = ctx @ out_w.T + c
  Per-head score bias is a constant shift within each softmax row and cancels
  exactly; all other biases fold into c = out_w @ bv + out_b (host).

Device per core: stream its 4 batches once as TWO copies — natural [n,d] in
fp16 (for the pooled matmul) and transposed [d,n] in fp8e4 (for the scores
matmul; fp8 verified to 6.5e-3 rel err end-to-end, and the PE accepts a
fp16-stationary x fp8-moving matmul) — 37.8 MB total. 1024-token tiles, one
DMA per stream per tile (split across the sync and scalar HWDGE rings).
Scores via fp16 ws x fp8 tokT matmuls, exp on ACT with accumulated row sums,
pooled as a PSUM-accumulated matmul against the natural tile. Per-batch
normalize + transpose overlap the next batch's stream; the final projections
are 36+14 matmuls straight into a [bloc, D] PSUM tile (no output transpose).
"""

import numpy as np

P = 128
D = 768
H = 12
DH = 64
DJ = D // P          # 6 chunks of the model dim
C = 1024             # tokens per tile
S = C // P           # 8 sub-chunks of 128 tokens
B = 32
N = 4096
NCORES = 8
BLOC = B // NCORES   # batches per core

_PATCHED = False


def _patch_tile_drain():
    """This walrus build allows only ONE sync wait per instruction (2 for
    EventSemaphore), but TileContext._drain_and_barrier puts a wait per
    outstanding semaphore on the single tail Drain. Split: one Drain each."""
    global _PATCHED
    if _PATCHED:
        return
    import bass_rust
    import concourse.tile as tile
    from concourse.vector_clock import ScopedClock

    def _drain_and_barrier(self, tick_clock, wait_clock):
        nc = self.nc
        probe = nc.sync.drain()
        wait_clock.add_sem_waits(
            probe.ins, ScopedClock({None: tick_clock.global_clock})
        )
        si = probe.ins.sync_info
        if si is not None and len(si.on_wait) > 1:
            waits = list(si.on_wait)
            probe.ins.sync_info = bass_rust.SyncInfo(
                on_wait=[waits[0]], on_update=list(si.on_update)
            )
            for w in waits[1:]:
                extra = nc.sync.drain()
                extra.ins.sync_info = bass_rust.SyncInfo(on_wait=[w], on_update=[])
        nc.all_engine_barrier()
        popped = nc._tile_sem_poison_stack.pop()
        assert popped is self._sem_poison
        nc.clear_and_free_semaphores(list(self.sems.allocated().values()))
        nc.all_engine_barrier()

    tile.TileContext._drain_and_barrier = _drain_and_barrier
    _PATCHED = True


def _legalize_waits(nc):
    """TRN2 walrus encodes at most ONE sync wait per instruction (two for
    EventSemaphore). Tile's wait assignment can leave more; hoist the extras
    onto standalone EventSemaphore instructions inserted just before, on the
    same engine (same semantics: engine blocks on them in order)."""
    import bass_rust
    from concourse import mybir

    n_fixed = 0
    for f in nc.m.functions:
        for bb in f.blocks:
            out = []
            for inst in bb.instructions:
                si = inst.sync_info
                waits = list(si.on_wait) if si is not None else []
                cap = 2 if isinstance(inst, mybir.InstEventSemaphore) else 1
                if len(waits) > cap:
                    extras, keep = waits[:-cap], waits[-cap:]
                    for i in range(0, len(extras), 2):
                        ev = mybir.InstEventSemaphore(
                            name=f"EVW-{inst.name}-{i}", ins=[], outs=[]
                        )
                        ev.engine = inst.engine
                        ev.sync_info = bass_rust.SyncInfo(
                            on_wait=extras[i : i + 2], on_update=[]
                        )
                        out.append(ev)
                    inst.sync_info = bass_rust.SyncInfo(
                        on_wait=keep, on_update=list(si.on_update)
                    )
                    n_fixed += 1
                out.append(inst)
            bb.instructions = out
    return n_fixed


def build_nc(bloc=BLOC, n=N, legalize=True):
    import concourse.bass as bass
    import concourse.tile as tile
    from concourse import mybir
    from concourse.masks import make_identity

    f32 = mybir.dt.float32
    f16 = mybir.dt.float16
    f8 = mybir.dt.float8e4
    EXP = mybir.ActivationFunctionType.Exp
    nt = n // C

    nc = bass.Bass()
    tokens = nc.declare_dram_parameter("tokens", [bloc, n, D], f16, isOutput=False)
    tokT8 = nc.declare_dram_parameter("tokT8", [bloc, D, n], f8, isOutput=False)
    wsT = nc.declare_dram_parameter("wsT", [DJ, P, H], f16, isOutput=False)
    wvT = nc.declare_dram_parameter("wvT", [DJ, P, D], f16, isOutput=False)
    owT = nc.declare_dram_parameter("owT", [DJ, P, D], f16, isOutput=False)
    cvec = nc.declare_dram_parameter("cvec", [1, D], f16, isOutput=False)
    ones = nc.declare_dram_parameter("ones", [1, bloc], f16, isOutput=False)
    out_d = nc.declare_dram_parameter("out", [bloc, D], f32, isOutput=True)

    tokens_ap = tokens[:, :, :]
    tokT8_ap = tokT8[:, :, :]

    with tile.TileContext(nc) as tc:
        with (
            tc.tile_pool(name="singles", bufs=1) as singles,
            tc.tile_pool(name="tok", bufs=4) as tok_pool,
            tc.tile_pool(name="tokT", bufs=4) as tokT_pool,
            tc.tile_pool(name="pp", bufs=2) as p_pool,
            tc.tile_pool(name="lp", bufs=2) as lp_pool,
            tc.tile_pool(name="pn", bufs=2) as pn_pool,
            tc.tile_pool(name="scps", bufs=4, space="PSUM") as sc_psum,
            tc.tile_pool(name="ptps", bufs=2, space="PSUM") as pt_psum,
            tc.tile_pool(name="pops", bufs=1, space="PSUM") as pooled_psum,
        ):
            flat = []
            for b in range(bloc):
                for t in range(nt):
                    flat.append((b, t * C, C, t == 0, t == nt - 1))

            def issue_tokT_dma(key):
                b, n0, sz, _, _ = key
                tokT = tokT_pool.tile([P, DJ, sz], f8, tag="tokT")
                nc.scalar.dma_start(
                    out=tokT,
                    in_=tokT8_ap[b, :, n0 : n0 + sz].rearrange(
                        "(j p) n -> p j n", p=P
                    ),
                )
                return tokT

            def issue_tok_dma(key):
                b, n0, sz, _, _ = key
                tok_h = tok_pool.tile([P, sz // P, D], f16, tag="tok")
                nc.sync.dma_start(
                    out=tok_h,
                    in_=tokens_ap[b, n0 : n0 + sz, :].rearrange(
                        "(s p) d -> p s d", p=P
                    ),
                )
                return tok_h

            def issue_tile_dmas(key):
                return issue_tok_dma(key), issue_tokT_dma(key)

            # pre-issue the first tiles' token DMAs before any setup so HBM
            # streaming starts immediately; the tokT streams go first since
            # only they gate the first scores matmuls
            dmas = {}
            pre_tokT = {key: issue_tokT_dma(key) for key in flat[:4]}
            for key in flat[:4]:
                dmas[key] = (issue_tok_dma(key), pre_tokT[key])

            ident = singles.tile([P, P], f32)
            make_identity(nc, ident)
            ident_h = singles.tile([P, P], f16)
            nc.vector.tensor_copy(out=ident_h, in_=ident)
            wsT_sb = singles.tile([P, DJ, H], f16)
            nc.gpsimd.dma_start(
                out=wsT_sb, in_=wsT[:, :, :].rearrange("j p h -> p j h")
            )
            wvT_sb = singles.tile([P, DJ, D], f16)
            nc.gpsimd.dma_start(
                out=wvT_sb, in_=wvT[:, :, :].rearrange("j p d -> p j d")
            )
            owT_sb = singles.tile([P, DJ, D], f16)
            nc.gpsimd.dma_start(
                out=owT_sb, in_=owT[:, :, :].rearrange("j p d -> p j d")
            )
            cvec_sb = singles.tile([1, D], f16)
            nc.gpsimd.dma_start(out=cvec_sb, in_=cvec[:, :])
            ones_sb = singles.tile([1, bloc], f16)
            nc.gpsimd.dma_start(out=ones_sb, in_=ones[:, :])
            l_acc = singles.tile([H, bloc], f32)
            linv = singles.tile([H, bloc], f32)
            pstack = singles.tile([P, DJ, H, bloc], f16)

            pooled_tiles = {}

            def tile_block(key, tok_tiles):
                """Full per-tile compute, in 512-token halves so the exp of
                one half hides under the scores matmuls of the next and the
                PE never stalls on the ACT latency."""
                b, n0, sz, first, last = key
                tok_h, tokT = tok_tiles
                nhalves = sz // 512
                if first:
                    pooled_tiles[b] = pooled_psum.tile(
                        [H, D], f32, tag="po", name=f"pooled_b{b}"
                    )
                pooled_ps = pooled_tiles[b]
                p_halves = []
                for half in range(nhalves):
                    sl = slice(half * 512, (half + 1) * 512)
                    ps = sc_psum.tile([H, 512], f32, tag="sc")
                    for j in range(DJ):
                        nc.tensor.matmul(
                            ps,
                            wsT_sb[:, j, :],
                            tokT[:, j, sl],
                            start=(j == 0),
                            stop=(j == DJ - 1),
                        )
                    p_t = p_pool.tile([H, 512], f16, tag="p")
                    lp = lp_pool.tile([H, 1], f32, tag="l")
                    nc.scalar.activation(out=p_t, in_=ps, func=EXP, accum_out=lp)
                    if first and half == 0:
                        nc.vector.tensor_copy(out=l_acc[:, b : b + 1], in_=lp)
                    else:
                        nc.vector.tensor_add(
                            out=l_acc[:, b : b + 1], in0=l_acc[:, b : b + 1], in1=lp
                        )
                    p_halves.append(p_t)
                for half in range(nhalves):
                    p_t = p_halves[half]
                    pT_ps = pt_psum.tile([P, 4 * H], f16, tag="pt")
                    for s in range(4):
                        nc.tensor.transpose(
                            pT_ps[:, s * H : (s + 1) * H],
                            p_t[:, s * P : (s + 1) * P],
                            ident_h[:H, :H],
                        )
                    pT = p_pool.tile([P, 4 * H], f16, tag="pT")
                    nc.vector.tensor_copy(out=pT, in_=pT_ps)
                    for s in range(4):
                        st = first and half == 0 and s == 0
                        sp = last and half == nhalves - 1 and s == 3
                        sg = half * 4 + s
                        nc.tensor.matmul(
                            pooled_ps[:, 0:512],
                            pT[:, s * H : (s + 1) * H],
                            tok_h[:, sg, 0:512],
                            start=st,
                            stop=sp,
                        )
                        nc.tensor.matmul(
                            pooled_ps[:, 512:768],
                            pT[:, s * H : (s + 1) * H],
                            tok_h[:, sg, 512:768],
                            start=st,
                            stop=sp,
                        )
                if last:
                    # normalize this batch + transpose into pstack
                    nc.vector.reciprocal(linv[:, b : b + 1], l_acc[:, b : b + 1])
                    pooled_n = pn_pool.tile([H, D], f16, tag="pn")
                    nc.vector.tensor_scalar_mul(
                        pooled_n, pooled_ps, linv[:, b : b + 1]
                    )
                    trp = pt_psum.tile([P, DJ * H], f16, tag="pt")
                    for j in range(DJ):
                        nc.tensor.transpose(
                            trp[:, j * H : (j + 1) * H],
                            pooled_n[:, j * P : (j + 1) * P],
                            ident_h[:H, :H],
                        )
                    nc.vector.tensor_copy(
                        out=pstack[:, :, :, b],
                        in_=trp[:, :].rearrange("p (j h) -> p j h", h=H),
                    )

            for key in flat:
                if key not in dmas:
                    dmas[key] = issue_tile_dmas(key)
                tile_block(key, dmas.pop(key))

            # ---- tail: ctx = wv-projection (select 2 heads per e-block),
            # then out = ctx-blocks^T @ ow^T + bias, directly in [bloc, D]
            ctx_sb = singles.tile([P, DJ, bloc], f16)
            for e in range(DJ):
                po = pt_psum.tile([P, H * bloc], f32, tag="pt")
                for j in range(DJ):
                    nc.tensor.matmul(
                        po,
                        wvT_sb[:, j, e * P : (e + 1) * P],
                        pstack[:, j, :, :],
                        start=(j == 0),
                        stop=(j == DJ - 1),
                    )
                h0, h1 = 2 * e, 2 * e + 1
                nc.vector.tensor_copy(
                    out=ctx_sb[0:DH, e, :], in_=po[0:DH, h0 * bloc : (h0 + 1) * bloc]
                )
                nc.vector.tensor_copy(
                    out=ctx_sb[DH:P, e, :], in_=po[DH:P, h1 * bloc : (h1 + 1) * bloc]
                )
            out_ps = pooled_psum.tile([bloc, D], f32, tag="po")
            for half, sl in ((0, slice(0, 512)), (1, slice(512, 768))):
                nc.tensor.matmul(
                    out_ps[:, sl], ones_sb, cvec_sb[:, sl], start=True, stop=False
                )
                for e in range(DJ):
                    nc.tensor.matmul(
                        out_ps[:, sl],
                        ctx_sb[:, e, :],
                        owT_sb[:, e, sl],
                        start=False,
                        stop=(e == DJ - 1),
                    )
            fin_sb = singles.tile([bloc, D], f32)
            nc.vector.tensor_copy(out=fin_sb, in_=out_ps)
            nc.sync.dma_start(out=out_d[:, :], in_=fin_sb)
    if legalize:
        _legalize_waits(nc)
    return nc


def host_prep(query, in_proj_w, in_proj_b, out_w, out_b):
    scale = 1.0 / np.sqrt(DH)
    wq, wk = in_proj_w[:D], in_proj_w[D : 2 * D]
    wv = in_proj_w[2 * D :]
    bq = in_proj_b[:D]
    bv = in_proj_b[2 * D :]
    q_flat = query[0, 0] @ wq.T + bq
    ws = (q_flat.reshape(H, DH)[:, :, None] * wk.reshape(H, DH, D)).sum(1)
    ws_scaled = (ws * scale).astype(np.float32)
    wsT_r = np.ascontiguousarray(ws_scaled.T.astype(np.float16)).reshape(DJ, P, H)
    wvT_r = np.ascontiguousarray(wv.T.astype(np.float16)).reshape(DJ, P, D)
    owT_r = np.ascontiguousarray(out_w.T.astype(np.float16)).reshape(DJ, P, D)
    cvec_r = (out_w @ bv + out_b).astype(np.float16).reshape(1, D)
    ones_r = np.ones((1, BLOC), dtype=np.float16)
    return wsT_r, wvT_r, owT_r, cvec_r, ones_r


def make_in_maps(tokens):
    import ml_dtypes

    tok16 = tokens.astype(np.float16)
    tok8T = np.ascontiguousarray(tokens.transpose(0, 2, 1)).astype(
        ml_dtypes.float8_e4m3
    )
    return tok16, tok8T


def kernel(tokens, query, in_proj_w, in_proj_b, out_w, out_b):
    _patch_tile_drain()
    from concourse.bass_utils import run_bass_kernel_spmd

    tokens = np.asarray(tokens, dtype=np.float32)
    query = np.asarray(query, dtype=np.float32)
    in_proj_w = np.asarray(in_proj_w, dtype=np.float32)
    in_proj_b = np.asarray(in_proj_b, dtype=np.float32)
    out_w = np.asarray(out_w, dtype=np.float32)
    out_b = np.asarray(out_b, dtype=np.float32)

    wsT_r, wvT_r, owT_r, cvec_r, ones_r = host_prep(
        query, in_proj_w, in_proj_b, out_w, out_b
    )
    nc = build_nc()
    tok16, tok8T = make_in_maps(tokens)
    in_maps = [
        {
            "tokens": np.ascontiguousarray(tok16[i * BLOC : (i + 1) * BLOC]),
            "tokT8": tok8T[i * BLOC : (i + 1) * BLOC],
            "wsT": wsT_r,
            "wvT": wvT_r,
            "owT": owT_r,
            "cvec": cvec_r,
            "ones": ones_r,
        }
        for i in range(NCORES)
    ]
    res = run_bass_kernel_spmd(nc, in_maps, core_ids=list(range(NCORES)))
    return np.concatenate(
        [res.results[i]["out"] for i in range(NCORES)], axis=0
    ).astype(np.float32)


# revision 15
# speedup vs baseline: 1.1736x; 1.0515x over previous
"""AttentionPool Trainium2 kernel (8-core SPMD, batch-sharded).

Math (algebraically folded from the reference):
  The single learned query collapses attention to:
    ws[h,:]   = sum_{d in head h} q_flat[h*64+d] * wk[h*64+d, :]   (host, tiny)
    s[b,h,n]  = tokens[b,n,:] @ (ws[h,:] * scale)                  (device)
    p         = exp(s)            (softmax shift cancels; |s| <~ 2, fp32-safe)
    pooled    = (p @ tokens) / sum_n p                             (device)
    ctx[b,hd] = wv[hd,:] @ pooled[b,h,:] ;  out = ctx @ out_w.T + c
  Per-head score bias is a constant shift within each softmax row and cancels
  exactly; all other biases fold into c = out_w @ bv + out_b (host).

Device per core: stream its 4 batches once as TWO copies — natural [n,d] in
fp16 (for the pooled matmul) and transposed [d,n] in fp8e4 (for the scores
matmul; fp8 verified to 6.5e-3 rel err end-to-end, and the PE accepts a
fp16-stationary x fp8-moving matmul) — 37.8 MB total. 1024-token tiles, one
DMA per stream per tile (split across the sync and scalar HWDGE rings).
Scores via fp16 ws x fp8 tokT matmuls, exp on ACT with accumulated row sums,
pooled as a PSUM-accumulated matmul against the natural tile. Per-batch
normalize + transpose overlap the next batch's stream; the final projections
are 36+14 matmuls straight into a [bloc, D] PSUM tile (no output transpose).
"""

import numpy as np

P = 128
D = 768
H = 12
DH = 64
DJ = D // P          # 6 chunks of the model dim
C = 1024             # tokens per tile
S = C // P           # 8 sub-chunks of 128 tokens
B = 32
N = 4096
NCORES = 8
BLOC = B // NCORES   # batches per core

_PATCHED = False


def _patch_tile_drain():
    """This walrus build allows only ONE sync wait per instruction (2 for
    EventSemaphore), but TileContext._drain_and_barrier puts a wait per
    outstanding semaphore on the single tail Drain. Split: one Drain each."""
    global _PATCHED
    if _PATCHED:
        return
    import bass_rust
    import concourse.tile as tile
    from concourse.vector_clock import ScopedClock

    def _drain_and_barrier(self, tick_clock, wait_clock):
        nc = self.nc
        probe = nc.sync.drain()
        wait_clock.add_sem_waits(
            probe.ins, ScopedClock({None: tick_clock.global_clock})
        )
        si = probe.ins.sync_info
        if si is not None and len(si.on_wait) > 1:
            waits = list(si.on_wait)
            probe.ins.sync_info = bass_rust.SyncInfo(
                on_wait=[waits[0]], on_update=list(si.on_update)
            )
            for w in waits[1:]:
                extra = nc.sync.drain()
                extra.ins.sync_info = bass_rust.SyncInfo(on_wait=[w], on_update=[])
        nc.all_engine_barrier()
        popped = nc._tile_sem_poison_stack.pop()
        assert popped is self._sem_poison
        nc.clear_and_free_semaphores(list(self.sems.allocated().values()))
        nc.all_engine_barrier()

    tile.TileContext._drain_and_barrier = _drain_and_barrier
    _PATCHED = True


def _legalize_waits(nc):
    """TRN2 walrus encodes at most ONE sync wait per instruction (two for
    EventSemaphore). Tile's wait assignment can leave more; hoist the extras
    onto standalone EventSemaphore instructions inserted just before, on the
    same engine (same semantics: engine blocks on them in order)."""
    import bass_rust
    from concourse import mybir

    n_fixed = 0
    for f in nc.m.functions:
        for bb in f.blocks:
            out = []
            for inst in bb.instructions:
                si = inst.sync_info
                waits = list(si.on_wait) if si is not None else []
                cap = 2 if isinstance(inst, mybir.InstEventSemaphore) else 1
                if len(waits) > cap:
                    extras, keep = waits[:-cap], waits[-cap:]
                    for i in range(0, len(extras), 2):
                        ev = mybir.InstEventSemaphore(
                            name=f"EVW-{inst.name}-{i}", ins=[], outs=[]
                        )
                        ev.engine = inst.engine
                        ev.sync_info = bass_rust.SyncInfo(
                            on_wait=extras[i : i + 2], on_update=[]
                        )
                        out.append(ev)
                    inst.sync_info = bass_rust.SyncInfo(
                        on_wait=keep, on_update=list(si.on_update)
                    )
                    n_fixed += 1
                out.append(inst)
            bb.instructions = out
    return n_fixed


def build_nc(bloc=BLOC, n=N, legalize=True):
    import concourse.bass as bass
    import concourse.tile as tile
    from concourse import mybir
    from concourse.masks import make_identity

    f32 = mybir.dt.float32
    f16 = mybir.dt.float16
    f8 = mybir.dt.float8e4
    EXP = mybir.ActivationFunctionType.Exp
    nt = n // C

    nc = bass.Bass()
    tokens = nc.declare_dram_parameter("tokens", [bloc, n, D], f16, isOutput=False)
    tokT8 = nc.declare_dram_parameter("tokT8", [bloc, D, n], f8, isOutput=False)
    wsT = nc.declare_dram_parameter("wsT", [DJ, P, H], f16, isOutput=False)
    wvT = nc.declare_dram_parameter("wvT", [DJ, P, D], f16, isOutput=False)
    owT = nc.declare_dram_parameter("owT", [DJ, P, D], f16, isOutput=False)
    cvec = nc.declare_dram_parameter("cvec", [1, D], f16, isOutput=False)
    ones = nc.declare_dram_parameter("ones", [1, bloc], f16, isOutput=False)
    out_d = nc.declare_dram_parameter("out", [bloc, D], f32, isOutput=True)

    tokens_ap = tokens[:, :, :]
    tokT8_ap = tokT8[:, :, :]

    with tile.TileContext(nc) as tc:
        with (
            tc.tile_pool(name="singles", bufs=1) as singles,
            tc.tile_pool(name="tok", bufs=4) as tok_pool,
            tc.tile_pool(name="tokT", bufs=4) as tokT_pool,
            tc.tile_pool(name="pp", bufs=2) as p_pool,
            tc.tile_pool(name="lp", bufs=2) as lp_pool,
            tc.tile_pool(name="pn", bufs=2) as pn_pool,
            tc.tile_pool(name="scps", bufs=4, space="PSUM") as sc_psum,
            tc.tile_pool(name="ptps", bufs=2, space="PSUM") as pt_psum,
            tc.tile_pool(name="pops", bufs=1, space="PSUM") as pooled_psum,
        ):
            flat = []
            for b in range(bloc):
                for t in range(nt):
                    flat.append((b, t * C, C, t == 0, t == nt - 1))

            def issue_tokT_dma(key):
                b, n0, sz, _, _ = key
                tokT = tokT_pool.tile([P, DJ, sz], f8, tag="tokT")
                nc.scalar.dma_start(
                    out=tokT,
                    in_=tokT8_ap[b, :, n0 : n0 + sz].rearrange(
                        "(j p) n -> p j n", p=P
                    ),
                )
                return tokT

            def issue_tok_dma(key):
                b, n0, sz, _, _ = key
                tok_h = tok_pool.tile([P, sz // P, D], f16, tag="tok")
                nc.sync.dma_start(
                    out=tok_h,
                    in_=tokens_ap[b, n0 : n0 + sz, :].rearrange(
                        "(s p) d -> p s d", p=P
                    ),
                )
                return tok_h

            def issue_tile_dmas(key):
                return issue_tok_dma(key), issue_tokT_dma(key)

            # pre-issue the first tiles' token DMAs before any setup so HBM
            # streaming starts immediately
            dmas = {}
            for key in flat[:4]:
                dmas[key] = issue_tile_dmas(key)

            ident = singles.tile([P, P], f32)
            make_identity(nc, ident)
            ident_h = singles.tile([P, P], f16)
            nc.vector.tensor_copy(out=ident_h, in_=ident)
            wsT_sb = singles.tile([P, DJ, H], f16)
            nc.gpsimd.dma_start(
                out=wsT_sb, in_=wsT[:, :, :].rearrange("j p h -> p j h")
            )
            wvT_sb = singles.tile([P, DJ, D], f16)
            nc.gpsimd.dma_start(
                out=wvT_sb, in_=wvT[:, :, :].rearrange("j p d -> p j d")
            )
            owT_sb = singles.tile([P, DJ, D], f16)
            nc.gpsimd.dma_start(
                out=owT_sb, in_=owT[:, :, :].rearrange("j p d -> p j d")
            )
            cvec_sb = singles.tile([1, D], f16)
            nc.gpsimd.dma_start(out=cvec_sb, in_=cvec[:, :])
            ones_sb = singles.tile([1, bloc], f16)
            nc.gpsimd.dma_start(out=ones_sb, in_=ones[:, :])
            l_acc = singles.tile([H, bloc], f32)
            linv = singles.tile([H, bloc], f32)
            pstack = singles.tile([P, DJ, H, bloc], f16)

            pooled_tiles = {}

            def tile_block(key, tok_tiles):
                """Full per-tile compute, in 512-token halves so the exp of
                one half hides under the scores matmuls of the next and the
                PE never stalls on the ACT latency."""
                b, n0, sz, first, last = key
                tok_h, tokT = tok_tiles
                nhalves = sz // 512
                if first:
                    pooled_tiles[b] = pooled_psum.tile(
                        [H, D], f32, tag="po", name=f"pooled_b{b}"
                    )
                pooled_ps = pooled_tiles[b]
                p_halves = []
                for half in range(nhalves):
                    sl = slice(half * 512, (half + 1) * 512)
                    ps = sc_psum.tile([H, 512], f32, tag="sc")
                    for j in range(DJ):
                        nc.tensor.matmul(
                            ps,
                            wsT_sb[:, j, :],
                            tokT[:, j, sl],
                            start=(j == 0),
                            stop=(j == DJ - 1),
                        )
                    p_t = p_pool.tile([H, 512], f16, tag="p")
                    lp = lp_pool.tile([H, 1], f32, tag="l")
                    nc.scalar.activation(out=p_t, in_=ps, func=EXP, accum_out=lp)
                    if first and half == 0:
                        nc.vector.tensor_copy(out=l_acc[:, b : b + 1], in_=lp)
                    else:
                        nc.vector.tensor_add(
                            out=l_acc[:, b : b + 1], in0=l_acc[:, b : b + 1], in1=lp
                        )
                    p_halves.append(p_t)
                for half in range(nhalves):
                    p_t = p_halves[half]
                    pT_ps = pt_psum.tile([P, 4 * H], f16, tag="pt")
                    for s in range(4):
                        nc.tensor.transpose(
                            pT_ps[:, s * H : (s + 1) * H],
                            p_t[:, s * P : (s + 1) * P],
                            ident_h[:H, :H],
                        )
                    pT = p_pool.tile([P, 4 * H], f16, tag="pT")
                    nc.vector.tensor_copy(out=pT, in_=pT_ps)
                    for s in range(4):
                        st = first and half == 0 and s == 0
                        sp = last and half == nhalves - 1 and s == 3
                        sg = half * 4 + s
                        nc.tensor.matmul(
                            pooled_ps[:, 0:512],
                            pT[:, s * H : (s + 1) * H],
                            tok_h[:, sg, 0:512],
                            start=st,
                            stop=sp,
                        )
                        nc.tensor.matmul(
                            pooled_ps[:, 512:768],
                            pT[:, s * H : (s + 1) * H],
                            tok_h[:, sg, 512:768],
                            start=st,
                            stop=sp,
                        )
                if last:
                    # normalize this batch + transpose into pstack
                    nc.vector.reciprocal(linv[:, b : b + 1], l_acc[:, b : b + 1])
                    pooled_n = pn_pool.tile([H, D], f16, tag="pn")
                    nc.vector.tensor_scalar_mul(
                        pooled_n, pooled_ps, linv[:, b : b + 1]
                    )
                    trp = pt_psum.tile([P, DJ * H], f16, tag="pt")
                    for j in range(DJ):
                        nc.tensor.transpose(
                            trp[:, j * H : (j + 1) * H],
                            pooled_n[:, j * P : (j + 1) * P],
                            ident_h[:H, :H],
                        )
                    nc.vector.tensor_copy(
                        out=pstack[:, :, :, b],
                        in_=trp[:, :].rearrange("p (j h) -> p j h", h=H),
                    )

            for key in flat:
                if key not in dmas:
                    dmas[key] = issue_tile_dmas(key)
                tile_block(key, dmas.pop(key))

            # ---- tail: ctx = wv-projection (select 2 heads per e-block),
            # then out = ctx-blocks^T @ ow^T + bias, directly in [bloc, D]
            ctx_sb = singles.tile([P, DJ, bloc], f16)
            for e in range(DJ):
                po = pt_psum.tile([P, H * bloc], f32, tag="pt")
                for j in range(DJ):
                    nc.tensor.matmul(
                        po,
                        wvT_sb[:, j, e * P : (e + 1) * P],
                        pstack[:, j, :, :],
                        start=(j == 0),
                        stop=(j == DJ - 1),
                    )
                h0, h1 = 2 * e, 2 * e + 1
                nc.vector.tensor_copy(
                    out=ctx_sb[0:DH, e, :], in_=po[0:DH, h0 * bloc : (h0 + 1) * bloc]
                )
                nc.vector.tensor_copy(
                    out=ctx_sb[DH:P, e, :], in_=po[DH:P, h1 * bloc : (h1 + 1) * bloc]
                )
            out_ps = pooled_psum.tile([bloc, D], f32, tag="po")
            for half, sl in ((0, slice(0, 512)), (1, slice(512, 768))):
                nc.tensor.matmul(
                    out_ps[:, sl], ones_sb, cvec_sb[:, sl], start=True, stop=False
                )
                for e in range(DJ):
                    nc.tensor.matmul(
                        out_ps[:, sl],
                        ctx_sb[:, e, :],
                        owT_sb[:, e, sl],
                        start=False,
                        stop=(e == DJ - 1),
                    )
            fin_sb = singles.tile([bloc, D], f32)
            nc.vector.tensor_copy(out=fin_sb, in_=out_ps)
            nc.sync.dma_start(out=out_d[:, :], in_=fin_sb)
    if legalize:
        _legalize_waits(nc)
    return nc


def host_prep(query, in_proj_w, in_proj_b, out_w, out_b):
    scale = 1.0 / np.sqrt(DH)
    wq, wk = in_proj_w[:D], in_proj_w[D : 2 * D]
    wv = in_proj_w[2 * D :]
    bq = in_proj_b[:D]
    bv = in_proj_b[2 * D :]
    q_flat = query[0, 0] @ wq.T + bq
    ws = (q_flat.reshape(H, DH)[:, :, None] * wk.reshape(H, DH, D)).sum(1)
    ws_scaled = (ws * scale).astype(np.float32)
    wsT_r = np.ascontiguousarray(ws_scaled.T.astype(np.float16)).reshape(DJ, P, H)
    wvT_r = np.ascontiguousarray(wv.T.astype(np.float16)).reshape(DJ, P, D)
    owT_r = np.ascontiguousarray(out_w.T.astype(np.float16)).reshape(DJ, P, D)
    cvec_r = (out_w @ bv + out_b).astype(np.float16).reshape(1, D)
    ones_r = np.ones((1, BLOC), dtype=np.float16)
    return wsT_r, wvT_r, owT_r, cvec_r, ones_r


def make_in_maps(tokens):
    import ml_dtypes

    tok16 = tokens.astype(np.float16)
    tok8T = np.ascontiguousarray(tokens.transpose(0, 2, 1)).astype(
        ml_dtypes.float8_e4m3
    )
    return tok16, tok8T


def kernel(tokens, query, in_proj_w, in_proj_b, out_w, out_b):
    _patch_tile_drain()
    from concourse.bass_utils import run_bass_kernel_spmd

    tokens = np.asarray(tokens, dtype=np.float32)
    query = np.asarray(query, dtype=np.float32)
    in_proj_w = np.asarray(in_proj_w, dtype=np.float32)
    in_proj_b = np.asarray(in_proj_b, dtype=np.float32)
    out_w = np.asarray(out_w, dtype=np.float32)
    out_b = np.asarray(out_b, dtype=np.float32)

    wsT_r, wvT_r, owT_r, cvec_r, ones_r = host_prep(
        query, in_proj_w, in_proj_b, out_w, out_b
    )
    nc = build_nc()
    tok16, tok8T = make_in_maps(tokens)
    in_maps = [
        {
            "tokens": np.ascontiguousarray(tok16[i * BLOC : (i + 1) * BLOC]),
            "tokT8": tok8T[i * BLOC : (i + 1) * BLOC],
            "wsT": wsT_r,
            "wvT": wvT_r,
            "owT": owT_r,
            "cvec": cvec_r,
            "ones": ones_r,
        }
        for i in range(NCORES)
    ]
    res = run_bass_kernel_spmd(nc, in_maps, core_ids=list(range(NCORES)))
    return np.concatenate(
        [res.results[i]["out"] for i in range(NCORES)], axis=0
    ).astype(np.float32)


# revision 17
# speedup vs baseline: 1.1737x; 1.0000x over previous
"""AttentionPool Trainium2 kernel (8-core SPMD, batch-sharded).

Math (algebraically folded from the reference):
  The single learned query collapses attention to:
    ws[h,:]   = sum_{d in head h} q_flat[h*64+d] * wk[h*64+d, :]   (host, tiny)
    s[b,h,n]  = tokens[b,n,:] @ (ws[h,:] * scale)                  (device)
    p         = exp(s)            (softmax shift cancels; |s| <~ 2, fp32-safe)
    pooled    = (p @ tokens) / sum_n p                             (device)
    ctx[b,hd] = wv[hd,:] @ pooled[b,h,:] ;  out = ctx @ out_w.T + c
  Per-head score bias is a constant shift within each softmax row and cancels
  exactly; all other biases fold into c = out_w @ bv + out_b (host).

Device per core: stream its 4 batches once as TWO copies — natural [n,d] in
fp16 (for the pooled matmul) and transposed [d,n] in fp8e4 (for the scores
matmul; fp8 verified to 6.5e-3 rel err end-to-end, and the PE accepts a
fp16-stationary x fp8-moving matmul) — 37.8 MB total. 1024-token tiles, one
DMA per stream per tile (split across the sync and scalar HWDGE rings).
Scores via fp16 ws x fp8 tokT matmuls, exp on ACT with accumulated row sums,
pooled as a PSUM-accumulated matmul against the natural tile. Per-batch
normalize + transpose overlap the next batch's stream; the final projections
are 36+14 matmuls straight into a [bloc, D] PSUM tile (no output transpose).
"""

import numpy as np

P = 128
D = 768
H = 12
DH = 64
DJ = D // P          # 6 chunks of the model dim
C = 1024             # tokens per tile
S = C // P           # 8 sub-chunks of 128 tokens
B = 32
N = 4096
NCORES = 8
BLOC = B // NCORES   # batches per core

_PATCHED = False


def _patch_tile_drain():
    """This walrus build allows only ONE sync wait per instruction (2 for
    EventSemaphore), but TileContext._drain_and_barrier puts a wait per
    outstanding semaphore on the single tail Drain. Split: one Drain each."""
    global _PATCHED
    if _PATCHED:
        return
    import bass_rust
    import concourse.tile as tile
    from concourse.vector_clock import ScopedClock

    def _drain_and_barrier(self, tick_clock, wait_clock):
        nc = self.nc
        probe = nc.sync.drain()
        wait_clock.add_sem_waits(
            probe.ins, ScopedClock({None: tick_clock.global_clock})
        )
        si = probe.ins.sync_info
        if si is not None and len(si.on_wait) > 1:
            waits = list(si.on_wait)
            probe.ins.sync_info = bass_rust.SyncInfo(
                on_wait=[waits[0]], on_update=list(si.on_update)
            )
            for w in waits[1:]:
                extra = nc.sync.drain()
                extra.ins.sync_info = bass_rust.SyncInfo(on_wait=[w], on_update=[])
        nc.all_engine_barrier()
        popped = nc._tile_sem_poison_stack.pop()
        assert popped is self._sem_poison
        nc.clear_and_free_semaphores(list(self.sems.allocated().values()))
        nc.all_engine_barrier()

    tile.TileContext._drain_and_barrier = _drain_and_barrier
    _PATCHED = True


def _legalize_waits(nc):
    """TRN2 walrus encodes at most ONE sync wait per instruction (two for
    EventSemaphore). Tile's wait assignment can leave more; hoist the extras
    onto standalone EventSemaphore instructions inserted just before, on the
    same engine (same semantics: engine blocks on them in order)."""
    import bass_rust
    from concourse import mybir

    n_fixed = 0
    for f in nc.m.functions:
        for bb in f.blocks:
            out = []
            for inst in bb.instructions:
                si = inst.sync_info
                waits = list(si.on_wait) if si is not None else []
                cap = 2 if isinstance(inst, mybir.InstEventSemaphore) else 1
                if len(waits) > cap:
                    extras, keep = waits[:-cap], waits[-cap:]
                    for i in range(0, len(extras), 2):
                        ev = mybir.InstEventSemaphore(
                            name=f"EVW-{inst.name}-{i}", ins=[], outs=[]
                        )
                        ev.engine = inst.engine
                        ev.sync_info = bass_rust.SyncInfo(
                            on_wait=extras[i : i + 2], on_update=[]
                        )
                        out.append(ev)
                    inst.sync_info = bass_rust.SyncInfo(
                        on_wait=keep, on_update=list(si.on_update)
                    )
                    n_fixed += 1
                out.append(inst)
            bb.instructions = out
    return n_fixed


def build_nc(bloc=BLOC, n=N, legalize=True):
    import concourse.bass as bass
    import concourse.tile as tile
    from concourse import mybir
    from concourse.masks import make_identity

    f32 = mybir.dt.float32
    f16 = mybir.dt.float16
    f8 = mybir.dt.float8e4
    EXP = mybir.ActivationFunctionType.Exp
    nt = n // C

    nc = bass.Bass()
    tokens = nc.declare_dram_parameter("tokens", [bloc, n, D], f16, isOutput=False)
    tokT8 = nc.declare_dram_parameter("tokT8", [bloc, D, n], f8, isOutput=False)
    wsT = nc.declare_dram_parameter("wsT", [DJ, P, H], f16, isOutput=False)
    wvT = nc.declare_dram_parameter("wvT", [DJ, P, D], f16, isOutput=False)
    owT = nc.declare_dram_parameter("owT", [DJ, P, D], f16, isOutput=False)
    cvec = nc.declare_dram_parameter("cvec", [1, D], f16, isOutput=False)
    ones = nc.declare_dram_parameter("ones", [1, bloc], f16, isOutput=False)
    out_d = nc.declare_dram_parameter("out", [bloc, D], f32, isOutput=True)

    tokens_ap = tokens[:, :, :]
    tokT8_ap = tokT8[:, :, :]

    with tile.TileContext(nc) as tc:
        with (
            tc.tile_pool(name="singles", bufs=1) as singles,
            tc.tile_pool(name="tok", bufs=4) as tok_pool,
            tc.tile_pool(name="tokT", bufs=4) as tokT_pool,
            tc.tile_pool(name="pp", bufs=2) as p_pool,
            tc.tile_pool(name="lp", bufs=2) as lp_pool,
            tc.tile_pool(name="pn", bufs=2) as pn_pool,
            tc.tile_pool(name="scps", bufs=4, space="PSUM") as sc_psum,
            tc.tile_pool(name="ptps", bufs=2, space="PSUM") as pt_psum,
            tc.tile_pool(name="pops", bufs=1, space="PSUM") as pooled_psum,
        ):
            flat = []
            for b in range(bloc):
                for t in range(nt):
                    flat.append((b, t * C, C, t == 0, t == nt - 1))

            def issue_tokT_dma(key):
                b, n0, sz, _, _ = key
                tokT = tokT_pool.tile([P, DJ, sz], f8, tag="tokT")
                nc.scalar.dma_start(
                    out=tokT,
                    in_=tokT8_ap[b, :, n0 : n0 + sz].rearrange(
                        "(j p) n -> p j n", p=P
                    ),
                )
                return tokT

            def issue_tok_dma(key):
                b, n0, sz, _, _ = key
                tok_h = tok_pool.tile([P, sz // P, D], f16, tag="tok")
                nc.sync.dma_start(
                    out=tok_h,
                    in_=tokens_ap[b, n0 : n0 + sz, :].rearrange(
                        "(s p) d -> p s d", p=P
                    ),
                )
                return tok_h

            def issue_tile_dmas(key):
                return issue_tok_dma(key), issue_tokT_dma(key)

            # pre-issue the first tiles' token DMAs before any setup so HBM
            # streaming starts immediately
            dmas = {}
            for key in flat[:4]:
                dmas[key] = issue_tile_dmas(key)

            ident = singles.tile([P, P], f32)
            make_identity(nc, ident)
            ident_h = singles.tile([P, P], f16)
            nc.vector.tensor_copy(out=ident_h, in_=ident)
            wsT_sb = singles.tile([P, DJ, H], f16)
            nc.gpsimd.dma_start(
                out=wsT_sb, in_=wsT[:, :, :].rearrange("j p h -> p j h")
            )
            cvec_sb = singles.tile([1, D], f16)
            nc.gpsimd.dma_start(out=cvec_sb, in_=cvec[:, :])
            ones_sb = singles.tile([1, bloc], f16)
            nc.gpsimd.dma_start(out=ones_sb, in_=ones[:, :])
            # the tail-projection weights (2.4 MB) are only needed after the
            # stream; issuing them late keeps early HBM bandwidth for the
            # first token tiles
            wvT_sb = singles.tile([P, DJ, D], f16)
            owT_sb = singles.tile([P, DJ, D], f16)

            def issue_tail_weights():
                nc.gpsimd.dma_start(
                    out=wvT_sb, in_=wvT[:, :, :].rearrange("j p d -> p j d")
                )
                nc.gpsimd.dma_start(
                    out=owT_sb, in_=owT[:, :, :].rearrange("j p d -> p j d")
                )
            l_acc = singles.tile([H, bloc], f32)
            linv = singles.tile([H, bloc], f32)
            pstack = singles.tile([P, DJ, H, bloc], f16)

            pooled_tiles = {}

            def tile_block(key, tok_tiles):
                """Full per-tile compute, in 512-token halves so the exp of
                one half hides under the scores matmuls of the next and the
                PE never stalls on the ACT latency."""
                b, n0, sz, first, last = key
                tok_h, tokT = tok_tiles
                nhalves = sz // 512
                if first:
                    pooled_tiles[b] = pooled_psum.tile(
                        [H, D], f32, tag="po", name=f"pooled_b{b}"
                    )
                pooled_ps = pooled_tiles[b]
                p_halves = []
                for half in range(nhalves):
                    sl = slice(half * 512, (half + 1) * 512)
                    ps = sc_psum.tile([H, 512], f32, tag="sc")
                    for j in range(DJ):
                        nc.tensor.matmul(
                            ps,
                            wsT_sb[:, j, :],
                            tokT[:, j, sl],
                            start=(j == 0),
                            stop=(j == DJ - 1),
                        )
                    p_t = p_pool.tile([H, 512], f16, tag="p")
                    lp = lp_pool.tile([H, 1], f32, tag="l")
                    nc.scalar.activation(out=p_t, in_=ps, func=EXP, accum_out=lp)
                    if first and half == 0:
                        nc.vector.tensor_copy(out=l_acc[:, b : b + 1], in_=lp)
                    else:
                        nc.vector.tensor_add(
                            out=l_acc[:, b : b + 1], in0=l_acc[:, b : b + 1], in1=lp
                        )
                    p_halves.append(p_t)
                for half in range(nhalves):
                    p_t = p_halves[half]
                    pT_ps = pt_psum.tile([P, 4 * H], f16, tag="pt")
                    for s in range(4):
                        nc.tensor.transpose(
                            pT_ps[:, s * H : (s + 1) * H],
                            p_t[:, s * P : (s + 1) * P],
                            ident_h[:H, :H],
                        )
                    pT = p_pool.tile([P, 4 * H], f16, tag="pT")
                    nc.vector.tensor_copy(out=pT, in_=pT_ps)
                    for s in range(4):
                        st = first and half == 0 and s == 0
                        sp = last and half == nhalves - 1 and s == 3
                        sg = half * 4 + s
                        nc.tensor.matmul(
                            pooled_ps[:, 0:512],
                            pT[:, s * H : (s + 1) * H],
                            tok_h[:, sg, 0:512],
                            start=st,
                            stop=sp,
                        )
                        nc.tensor.matmul(
                            pooled_ps[:, 512:768],
                            pT[:, s * H : (s + 1) * H],
                            tok_h[:, sg, 512:768],
                            start=st,
                            stop=sp,
                        )
                if last:
                    # normalize this batch + transpose into pstack
                    nc.vector.reciprocal(linv[:, b : b + 1], l_acc[:, b : b + 1])
                    pooled_n = pn_pool.tile([H, D], f16, tag="pn")
                    nc.vector.tensor_scalar_mul(
                        pooled_n, pooled_ps, linv[:, b : b + 1]
                    )
                    trp = pt_psum.tile([P, DJ * H], f16, tag="pt")
                    for j in range(DJ):
                        nc.tensor.transpose(
                            trp[:, j * H : (j + 1) * H],
                            pooled_n[:, j * P : (j + 1) * P],
                            ident_h[:H, :H],
                        )
                    nc.vector.tensor_copy(
                        out=pstack[:, :, :, b],
                        in_=trp[:, :].rearrange("p (j h) -> p j h", h=H),
                    )

            for i, key in enumerate(flat):
                if i == 6:
                    issue_tail_weights()
                if key not in dmas:
                    dmas[key] = issue_tile_dmas(key)
                tile_block(key, dmas.pop(key))

            # ---- tail: ctx = wv-projection (select 2 heads per e-block),
            # then out = ctx-blocks^T @ ow^T + bias, directly in [bloc, D]
            ctx_sb = singles.tile([P, DJ, bloc], f16)
            for e in range(DJ):
                po = pt_psum.tile([P, H * bloc], f32, tag="pt")
                for j in range(DJ):
                    nc.tensor.matmul(
                        po,
                        wvT_sb[:, j, e * P : (e + 1) * P],
                        pstack[:, j, :, :],
                        start=(j == 0),
                        stop=(j == DJ - 1),
                    )
                h0, h1 = 2 * e, 2 * e + 1
                nc.vector.tensor_copy(
                    out=ctx_sb[0:DH, e, :], in_=po[0:DH, h0 * bloc : (h0 + 1) * bloc]
                )
                nc.vector.tensor_copy(
                    out=ctx_sb[DH:P, e, :], in_=po[DH:P, h1 * bloc : (h1 + 1) * bloc]
                )
            out_ps = pooled_psum.tile([bloc, D], f32, tag="po")
            for half, sl in ((0, slice(0, 512)), (1, slice(512, 768))):
                nc.tensor.matmul(
                    out_ps[:, sl], ones_sb, cvec_sb[:, sl], start=True, stop=False
                )
                for e in range(DJ):
                    nc.tensor.matmul(
                        out_ps[:, sl],
                        ctx_sb[:, e, :],
                        owT_sb[:, e, sl],
                        start=False,
                        stop=(e == DJ - 1),
                    )
            fin_sb = singles.tile([bloc, D], f32)
            nc.vector.tensor_copy(out=fin_sb, in_=out_ps)
            nc.sync.dma_start(out=out_d[:, :], in_=fin_sb)
    if legalize:
        _legalize_waits(nc)
    return nc


def host_prep(query, in_proj_w, in_proj_b, out_w, out_b):
    scale = 1.0 / np.sqrt(DH)
    wq, wk = in_proj_w[:D], in_proj_w[D : 2 * D]
    wv = in_proj_w[2 * D :]
    bq = in_proj_b[:D]
    bv = in_proj_b[2 * D :]
    q_flat = query[0, 0] @ wq.T + bq
    ws = (q_flat.reshape(H, DH)[:, :, None] * wk.reshape(H, DH, D)).sum(1)
    ws_scaled = (ws * scale).astype(np.float32)
    wsT_r = np.ascontiguousarray(ws_scaled.T.astype(np.float16)).reshape(DJ, P, H)
    wvT_r = np.ascontiguousarray(wv.T.astype(np.float16)).reshape(DJ, P, D)
    owT_r = np.ascontiguousarray(out_w.T.astype(np.float16)).reshape(DJ, P, D)
    cvec_r = (out_w @ bv + out_b).astype(np.float16).reshape(1, D)
    ones_r = np.ones((1, BLOC), dtype=np.float16)
    return wsT_r, wvT_r, owT_r, cvec_r, ones_r


def make_in_maps(tokens):
    import ml_dtypes

    tok16 = tokens.astype(np.float16)
    tok8T = np.ascontiguousarray(tokens.transpose(0, 2, 1)).astype(
        ml_dtypes.float8_e4m3
    )
    return tok16, tok8T


def kernel(tokens, query, in_proj_w, in_proj_b, out_w, out_b):
    _patch_tile_drain()
    from concourse.bass_utils import run_bass_kernel_spmd

    tokens = np.asarray(tokens, dtype=np.float32)
    query = np.asarray(query, dtype=np.float32)
    in_proj_w = np.asarray(in_proj_w, dtype=np.float32)
    in_proj_b = np.asarray(in_proj_b, dtype=np.float32)
    out_w = np.asarray(out_w, dtype=np.float32)
    out_b = np.asarray(out_b, dtype=np.float32)

    wsT_r, wvT_r, owT_r, cvec_r, ones_r = host_prep(
        query, in_proj_w, in_proj_b, out_w, out_b
    )
    nc = build_nc()
    tok16, tok8T = make_in_maps(tokens)
    in_maps = [
        {
            "tokens": np.ascontiguousarray(tok16[i * BLOC : (i + 1) * BLOC]),
            "tokT8": tok8T[i * BLOC : (i + 1) * BLOC],
            "wsT": wsT_r,
            "wvT": wvT_r,
            "owT": owT_r,
            "cvec": cvec_r,
            "ones": ones_r,
        }
        for i in range(NCORES)
    ]
    res = run_bass_kernel_spmd(nc, in_maps, core_ids=list(range(NCORES)))
    return np.concatenate(
        [res.results[i]["out"] for i in range(NCORES)], axis=0
    ).astype(np.float32)
